# revision 15
# baseline (speedup 1.0000x reference)
"""AKOrN layer (attention-coupled Kuramoto oscillators) on 8 TRN2 NeuronCores.

Sharding: B*H = 2*4 = 8 (batch, head) pairs -> one pair per core.
Each core computes its head's attention matrix E = exp(scores) entirely in
SBUF (never touches HBM), runs the 5 Kuramoto steps locally, then the four
cores of each batch AllGather their cos(phases) (32KB) and every core computes
the full output projection for its batch. Host picks core 0 -> batch 0,
core 4 -> batch 1.

Self-contained: hardcodes all shapes; only imports concourse from the
container's /opt/trn_rl_repo.
"""

import math
import sys

import numpy as np

for _p in ("/opt/trn_rl_repo",):
    if _p not in sys.path:
        sys.path.insert(0, _p)

# Problem constants (from the reference nn.Module)
B, N, D, H, O = 2, 1024, 256, 4, 8
DT, STEPS = 0.1, 5
DK = D // H            # 64 head dim
P = 128                # partitions
NT = N // P            # 8 token tiles
NCORES = 8
SW = 2 * O + 1         # stationary width per j-tile: [sin | cos | ones] = 17
PI = float(np.pi)
TWO_PI = float(2 * np.pi)

_CACHE = {}


def _build_nc():
    import concourse.bacc as bacc
    import concourse.tile as tile
    import concourse.mybir as mybir
    from concourse.masks import make_identity
    from concourse.tile_rust import add_dep_helper

    f32 = mybir.dt.float32
    bf16 = mybir.dt.bfloat16
    ALU = mybir.AluOpType
    ACT = mybir.ActivationFunctionType

    nc = bacc.Bacc(
        "TRN2",
        target_bir_lowering=False,
        debug=False,
        enable_asserts=False,
        num_devices=NCORES,
    )

    # Per-core external inputs (host pre-sliced / transposed)
    xT = nc.dram_tensor("xT", [D, N], bf16, kind="ExternalInput")         # x[b].T (bf16)
    wqT = nc.dram_tensor("wqT", [D, DK], bf16, kind="ExternalInput")      # Wq_h.T
    wkT = nc.dram_tensor("wkT", [D, DK], bf16, kind="ExternalInput")      # Wk_h.T
    wpT = nc.dram_tensor("wpT", [D, O], bf16, kind="ExternalInput")       # Wp_h.T
    bpr = nc.dram_tensor("bpr", [O, 1], f32, kind="ExternalInput")        # bp_h + pi
    csdt = nc.dram_tensor("csdt", [P, 1], f32, kind="ExternalInput")      # DT*cs
    dtom = nc.dram_tensor("dtom", [P, NT * O], f32, kind="ExternalInput")  # DT*om tiled
    wob = nc.dram_tensor("wob", [H * O + 1, D], bf16, kind="ExternalInput")  # [Wo.T; bo]
    outp = nc.dram_tensor("out", [N, D], f32, kind="ExternalOutput")

    groups = [[0, 1, 2, 3], [4, 5, 6, 7]]

    with tile.TileContext(nc) as tc:
        with (
            tc.tile_pool(name="const", bufs=1) as const,
            tc.tile_pool(name="data", bufs=1) as data,
            tc.tile_pool(name="work", bufs=2) as work,
            tc.tile_pool(name="ps2", bufs=2, space="PSUM") as ps2,
            tc.tile_pool(name="ps1", bufs=1, space="PSUM") as ps1,
            tc.tile_pool(name="dram", bufs=1, space="DRAM") as dram,
        ):
            # ---------- load inputs ----------
            xtb = data.tile([P, 2 * N], bf16)       # x.T, kt-major
            for kt in range(2):
                nc.sync.dma_start(xtb[:, kt * N:(kt + 1) * N], xT[kt * P:(kt + 1) * P, :])

            wq_s = const.tile([P, 2 * DK], bf16)
            wk_s = const.tile([P, 2 * DK], bf16)
            wp_s = const.tile([P, 2 * O], bf16)
            for kt in range(2):
                nc.sync.dma_start(wq_s[:, kt * DK:(kt + 1) * DK], wqT[kt * P:(kt + 1) * P, :])
                nc.sync.dma_start(wk_s[:, kt * DK:(kt + 1) * DK], wkT[kt * P:(kt + 1) * P, :])
                nc.sync.dma_start(wp_s[:, kt * O:(kt + 1) * O], wpT[kt * P:(kt + 1) * P, :])
            bpr_s = const.tile([O, 1], f32)
            nc.sync.dma_start(bpr_s[:, :], bpr[:, :])
            csdt_s = const.tile([P, 1], f32)
            nc.sync.dma_start(csdt_s[:, :], csdt[:, :])
            dtom_s = const.tile([P, NT * O], f32)
            nc.sync.dma_start(dtom_s[:, :], dtom[:, :])
            wob_s = const.tile([H * O + 1, D], bf16)
            nc.sync.dma_start(wob_s[:, :], wob[:, :])

            ident = const.tile([P, P], f32)
            make_identity(nc, ident[:, :])
            b_mpi = const.tile([P, 1], f32)
            nc.vector.memset(b_mpi[:, :], -PI)
            b_hpi = const.tile([P, 1], f32)
            nc.vector.memset(b_hpi[:, :], PI / 2)

            # ---------- collective warmup (absorbs first-call cost) ----------
            with tc.high_priority():
                agw_sb = const.tile([1, 8], f32)
                nc.vector.memset(agw_sb[:, :], 0.0)
                agw_in = dram.tile([1, 8], f32)
                agw_out = dram.tile([4, 8], f32)
                nc.gpsimd.dma_start(agw_in[:, :], agw_sb[:, :])
                nc.gpsimd.collective_compute(
                    "AllGather",
                    ALU.bypass,
                    replica_groups=groups,
                    ins=[agw_in[:, :].opt()],
                    outs=[agw_out[:, :].opt()],
                )

            # ---------- q/k projections (bf16) ----------
            qt = data.tile([DK, N], bf16)
            ktt = data.tile([DK, N], bf16)
            for dst, w_s in ((qt, wq_s), (ktt, wk_s)):
                for ib in range(2):
                    pq = ps2.tile([DK, 512], f32, tag="mm2")
                    for kt in range(2):
                        nc.tensor.matmul(
                            pq[:, :],
                            lhsT=w_s[:, kt * DK:(kt + 1) * DK],
                            rhs=xtb[:, kt * N + ib * 512: kt * N + (ib + 1) * 512],
                            start=(kt == 0),
                            stop=(kt == 1),
                        )
                    nc.vector.tensor_copy(dst[:, ib * 512:(ib + 1) * 512], pq[:, :])

            # ---------- initial phases: phT [o, i] -> transpose to natural ----------
            phtp = ps2.tile([O, N], f32, tag="big", bufs=1)
            for ib in range(2):
                for kt in range(2):
                    nc.tensor.matmul(
                        phtp[:, ib * 512:(ib + 1) * 512],
                        lhsT=wp_s[:, kt * O:(kt + 1) * O],
                        rhs=xtb[:, kt * N + ib * 512: kt * N + (ib + 1) * 512],
                        start=(kt == 0),
                        stop=(kt == 1),
                    )
            pht_sb = work.tile([O, N], f32, tag="sgt")
            # + (bp + pi) while evacuating
            nc.vector.tensor_scalar(pht_sb[:, :], phtp[:, :], bpr_s[:, :], None, ALU.add)
            php = ps1.tile([P, NT * O], f32, tag="pt", bufs=2)
            for it in range(NT):
                nc.tensor.transpose(
                    php[:, it * O:(it + 1) * O],
                    pht_sb[:, it * P:(it + 1) * P],
                    ident[0:O, 0:O],
                )
            # shifted representation: ph' = wrap(ph + pi) into [0, 2pi).
            # HW tensor ops have no mod; use compare-and-correct (single
            # wrap is enough: |ph0| < 2pi and per-step drift < 0.15).
            ph = data.tile([P, NT * O], f32)
            wge = work.tile([P, NT * O], f32, tag="wge")
            wlt = work.tile([P, NT * O], f32, tag="wlt")

            def wrap_inplace(t):
                nc.vector.tensor_scalar(wge[:, :], t[:, :], TWO_PI, None, ALU.is_ge)
                nc.vector.tensor_scalar(wlt[:, :], t[:, :], 0.0, None, ALU.is_lt)
                nc.vector.tensor_tensor(wge[:, :], wlt[:, :], wge[:, :], ALU.subtract)
                nc.vector.scalar_tensor_tensor(
                    t[:, :], wge[:, :], TWO_PI, t[:, :], ALU.mult, ALU.add)

            nc.vector.tensor_copy(ph[:, :], php[:, :])
            wrap_inplace(ph)

            # ---------- scores + exp -> E^T (bf16, [j_p, jt-major i]) ----------
            etb = data.tile([P, NT * N], bf16)
            exp_insts = []
            for jt in range(NT):
                psc = ps2.tile([P, N], f32, tag="big", bufs=1)
                for ib in range(2):
                    nc.tensor.matmul(
                        psc[:, ib * 512:(ib + 1) * 512],
                        lhsT=ktt[:, jt * P:(jt + 1) * P],
                        rhs=qt[:, ib * 512:(ib + 1) * 512],
                        start=True,
                        stop=True,
                    )
                e_i = nc.scalar.activation(etb[:, jt * N:(jt + 1) * N], psc[:, :],
                                           ACT.Exp, scale=1.0 / math.sqrt(DK))
                exp_insts.append(e_i)

            # ---------- stationary sin/cos/ones tiles (double-buffered) ----------
            scw_a = data.tile([P, NT * SW], bf16)
            scw_b = data.tile([P, NT * SW], bf16)
            scws = [scw_a, scw_b]
            scw3s = [t[:, :].rearrange("p (t w) -> p t w", w=SW) for t in scws]
            for t in scws:
                for jt in range(NT):
                    nc.vector.memset(t[:, jt * SW + 2 * O: (jt + 1) * SW], 1.0)
            s_view = scw3s[0][:, :, 0:O]
            c_view = scw3s[0][:, :, O:2 * O]

            ph3 = ph[:, :].rearrange("p (t o) -> p t o", o=O)
            tmp = work.tile([P, NT * O], f32, tag="tmp")
            tmp3 = tmp[:, :].rearrange("p (t o) -> p t o", o=O)

            # s = sin(ph'-pi); c = cos(ph'-pi) = sin(pi/2 - |ph'-pi|)
            nc.scalar.activation(s_view, ph3, ACT.Sin, bias=b_mpi[:, :], scale=1.0)
            nc.scalar.activation(tmp3, ph3, ACT.Abs, bias=b_mpi[:, :], scale=1.0)
            ci = nc.scalar.activation(c_view, tmp3, ACT.Sin, bias=b_hpi[:, :], scale=-1.0)
            # ACT stream grouped by table set: [init sins] -> [exps] -> [step sins]
            add_dep_helper(exp_insts[0].ins, ci.ins, sync=False,
                           reason="group ACT ops by table set")

            # ---------- Kuramoto steps ----------
            gfull = data.tile([P, NT * O], f32)
            gfull3 = gfull[:, :].rearrange("p (t o) -> p t o", o=O)
            rinv = data.tile([P, NT], f32)
            dtom3 = dtom_s[:, :].rearrange("p (t o) -> p t o", o=O)
            HB = NT // 2  # it-tiles per half
            cnat = data.tile([P, NT * O], bf16)
            cnat3 = cnat[:, :].rearrange("p (t o) -> p t o", o=O)

            for step in range(STEPS):
                scw = scws[step % 2]
                scw3 = scw3s[step % 2]
                scw3_nxt = scw3s[(step + 1) % 2]
                pt = ps1.tile([P, NT * SW + 2 * O], f32, tag="pt", bufs=2)
                pt3 = pt[:, 0:NT * SW].rearrange("p (t w) -> p t w", w=SW)
                for ib in range(2):
                    pc = ps2.tile([SW, 512], f32, tag="pc")
                    for jt in range(NT):
                        nc.tensor.matmul(
                            pc[:, :],
                            lhsT=scw[:, jt * SW:(jt + 1) * SW],
                            rhs=etb[:, jt * N + ib * 512: jt * N + (ib + 1) * 512],
                            start=(jt == 0),
                            stop=(jt == NT - 1),
                        )
                    ce = work.tile([SW, 512], f32, tag="ce")
                    nc.vector.tensor_copy(ce[:, :], pc[:, :])
                    for itl in range(HB):
                        it = ib * HB + itl
                        nc.tensor.transpose(
                            pt[:, it * SW:(it + 1) * SW],
                            ce[:, itl * P:(itl + 1) * P],
                            ident[0:SW, 0:SW],
                        )

                    # ---- per-half phase update (pipelines with other half) ----
                    hs = slice(ib * HB, (ib + 1) * HB)
                    es_v = pt3[:, hs, 0:O]
                    ec_v = pt3[:, hs, O:2 * O]
                    sv = scw3[:, hs, 0:O]
                    cv = scw3[:, hs, O:2 * O]
                    ph_h = ph3[:, hs, :]
                    if step == 0:
                        nc.vector.reciprocal(rinv[:, hs, None], pt3[:, hs, 2 * O:SW])
                        nc.vector.tensor_scalar(
                            gfull3[:, hs, :],
                            rinv[:, hs, None].to_broadcast((P, HB, O)),
                            csdt_s[:, :], None, ALU.mult,
                        )
                    t1 = work.tile([P, HB * O], f32, tag="t1")
                    t13 = t1[:, :].rearrange("p (t o) -> p t o", o=O)
                    t2 = work.tile([P, HB * O], f32, tag="t2")
                    t23 = t2[:, :].rearrange("p (t o) -> p t o", o=O)
                    nc.vector.tensor_tensor(t13, cv, es_v, ALU.mult)
                    nc.vector.tensor_tensor(t23, sv, ec_v, ALU.mult)
                    nc.vector.tensor_tensor(t13, t13, t23, ALU.subtract)
                    nc.vector.tensor_tensor(t13, t13, gfull3[:, hs, :], ALU.mult)
                    nc.vector.tensor_tensor(t13, t13, dtom3[:, hs, :], ALU.add)
                    nc.vector.tensor_tensor(ph_h, ph_h, t13, ALU.add)
                    # wrap into [0, 2pi): ph += 2pi*([ph<0] - [ph>=2pi])
                    nc.vector.tensor_scalar(t23, ph_h, TWO_PI, None, ALU.is_ge)
                    nc.vector.scalar_tensor_tensor(
                        t23, ph_h, 0.0, t23, ALU.is_lt, ALU.subtract)
                    nc.vector.scalar_tensor_tensor(
                        ph_h, t23, TWO_PI, ph_h, ALU.mult, ALU.add)
                    # keep-warm: tiny matmuls tied to the update chain so the
                    # PE's HAM window sees activity during the DVE/ACT tail
                    nc.tensor.matmul(pt[0:1, NT * SW:NT * SW + O],
                                     lhsT=t1[0:1, 0:1], rhs=t1[0:1, 0:O],
                                     start=True, stop=True)
                    nc.tensor.matmul(pt[0:1, NT * SW + O:NT * SW + 2 * O],
                                     lhsT=t2[0:1, 0:1], rhs=t2[0:1, 0:O],
                                     start=True, stop=True)
                    if step < STEPS - 1:
                        sv_n = scw3_nxt[:, hs, 0:O]
                        cv_n = scw3_nxt[:, hs, O:2 * O]
                        nc.scalar.activation(sv_n, ph_h, ACT.Sin, bias=b_mpi[:, :], scale=1.0)
                        nc.scalar.activation(tmp3[:, hs, :], ph_h, ACT.Abs,
                                             bias=b_mpi[:, :], scale=1.0)
                        nc.scalar.activation(cv_n, tmp3[:, hs, :], ACT.Sin,
                                             bias=b_hpi[:, :], scale=-1.0)
                    else:
                        # final sig = cos(phases), per half (starts AG sooner)
                        nc.scalar.activation(tmp3[:, hs, :], ph_h, ACT.Abs,
                                             bias=b_mpi[:, :], scale=1.0)
                        nc.scalar.activation(cnat3[:, hs, :], tmp3[:, hs, :], ACT.Sin,
                                             bias=b_hpi[:, :], scale=-1.0)

            # ---------- sig^T -> AllGather ----------
            identb = const.tile([P, P], bf16)
            nc.vector.tensor_copy(identb[:, :], ident[:, :])
            pst = ps2.tile([O, N], bf16, tag="big", bufs=1)
            for it in range(NT):
                nc.tensor.transpose(
                    pst[:, it * P:(it + 1) * P],
                    cnat[:, it * O:(it + 1) * O],
                    identb[:, :],
                )
            sgt = work.tile([O, N], bf16, tag="sgt2")
            nc.vector.tensor_copy(sgt[:, :], pst[:, :])

            ag_in = dram.tile([O, N], bf16)
            ag_out = dram.tile([H * O, N], bf16)
            nc.sync.dma_start(ag_in[:, :], sgt[:, :])
            nc.gpsimd.collective_compute(
                "AllGather",
                ALU.bypass,
                replica_groups=groups,
                ins=[ag_in[:, :].opt()],
                outs=[ag_out[:, :].opt()],
            )
            sgf = data.tile([H * O + 1, N], bf16)
            nc.sync.dma_start(sgf[0:H * O, :], ag_out[:, :])
            nc.vector.memset(sgf[H * O:H * O + 1, :], 1.0)

            # ---------- output projection ----------
            for it in range(NT):
                po = ps2.tile([P, D], f32, tag="mm2")
                nc.tensor.matmul(po[:, :], lhsT=sgf[:, it * P:(it + 1) * P],
                                 rhs=wob_s[:, :], start=True, stop=True)
                ot = work.tile([P, D], f32, tag="ot")
                nc.vector.tensor_copy(ot[:, :], po[:, :])
                nc.sync.dma_start(outp[it * P:(it + 1) * P, :], ot[:, :])

    nc.compile()
    return nc


def get_nc():
    if "nc" not in _CACHE:
        _CACHE["nc"] = _build_nc()
    return _CACHE["nc"]


def make_in_maps(x, Wq, Wk, Wp, bp, Wo, bo, omega, coupling_scale):
    import concourse.mybir as mybir

    bf16 = mybir.dt.np(mybir.dt.bfloat16)
    x = np.asarray(x, np.float32)
    Wq = np.asarray(Wq, np.float32)
    Wk = np.asarray(Wk, np.float32)
    Wp = np.asarray(Wp, np.float32)
    bp = np.asarray(bp, np.float32)
    Wo = np.asarray(Wo, np.float32)
    bo = np.asarray(bo, np.float32)
    omega = np.asarray(omega, np.float32)
    cs = float(np.asarray(coupling_scale, np.float32))

    wob_full = np.ascontiguousarray(
        np.concatenate([Wo.T, bo[None, :]], axis=0)).astype(bf16)
    csdt_full = np.full((P, 1), DT * cs, np.float32)

    in_maps = []
    for c in range(NCORES):
        b, h = c // H, c % H
        in_maps.append({
            "xT": np.ascontiguousarray(x[b].T).astype(bf16),
            "wqT": np.ascontiguousarray(Wq[h * DK:(h + 1) * DK, :].T).astype(bf16),
            "wkT": np.ascontiguousarray(Wk[h * DK:(h + 1) * DK, :].T).astype(bf16),
            "wpT": np.ascontiguousarray(Wp[h * O:(h + 1) * O, :].T).astype(bf16),
            "bpr": np.ascontiguousarray(
                (bp[h * O:(h + 1) * O] + np.pi)[:, None], np.float32),
            "csdt": csdt_full,
            "dtom": np.ascontiguousarray(
                np.tile((DT * omega[h])[None, :], (P, NT)), np.float32),
            "wob": wob_full,
        })
    return in_maps


def run_on_hw(in_maps, trace=False):
    from concourse.bass_utils import run_bass_kernel_spmd

    nc = get_nc()
    return run_bass_kernel_spmd(nc, in_maps, core_ids=list(range(NCORES)), trace=trace)


def kernel(x, Wq, Wk, Wp, bp, Wo, bo, omega, coupling_scale):
    in_maps = make_in_maps(x, Wq, Wk, Wp, bp, Wo, bo, omega, coupling_scale)
    res = run_on_hw(in_maps, trace=False)
    out = np.stack([res.results[0]["out"], res.results[H]["out"]], axis=0)
    return np.ascontiguousarray(out, np.float32)


# revision 17
# speedup vs baseline: 1.0501x; 1.0501x over previous
"""AKOrN layer (attention-coupled Kuramoto oscillators) on 8 TRN2 NeuronCores.

Sharding: B*H = 2*4 = 8 (batch, head) pairs -> one pair per core.
Each core computes its head's attention matrix E = exp(scores) entirely in
SBUF (never touches HBM), runs the 5 Kuramoto steps locally, then the four
cores of each batch AllGather their cos(phases) (32KB) and every core computes
the full output projection for its batch. Host picks core 0 -> batch 0,
core 4 -> batch 1.

Self-contained: hardcodes all shapes; only imports concourse from the
container's /opt/trn_rl_repo.
"""

import math
import sys

import numpy as np

for _p in ("/opt/trn_rl_repo",):
    if _p not in sys.path:
        sys.path.insert(0, _p)

# Problem constants (from the reference nn.Module)
B, N, D, H, O = 2, 1024, 256, 4, 8
DT, STEPS = 0.1, 5
DK = D // H            # 64 head dim
P = 128                # partitions
NT = N // P            # 8 token tiles
NCORES = 8
SW = 2 * O + 1         # stationary width per j-tile: [sin | cos | ones] = 17
PI = float(np.pi)
TWO_PI = float(2 * np.pi)

_CACHE = {}


def _build_nc():
    import concourse.bacc as bacc
    import concourse.tile as tile
    import concourse.mybir as mybir
    from concourse.masks import make_identity
    from concourse.tile_rust import add_dep_helper

    f32 = mybir.dt.float32
    bf16 = mybir.dt.bfloat16
    ALU = mybir.AluOpType
    ACT = mybir.ActivationFunctionType

    nc = bacc.Bacc(
        "TRN2",
        target_bir_lowering=False,
        debug=False,
        enable_asserts=False,
        num_devices=NCORES,
    )

    # Per-core external inputs (host pre-sliced / transposed)
    xT = nc.dram_tensor("xT", [D, N], bf16, kind="ExternalInput")         # x[b].T (bf16)
    wqT = nc.dram_tensor("wqT", [D, DK], bf16, kind="ExternalInput")      # Wq_h.T
    wkT = nc.dram_tensor("wkT", [D, DK], bf16, kind="ExternalInput")      # Wk_h.T
    wpT = nc.dram_tensor("wpT", [D, O], bf16, kind="ExternalInput")       # Wp_h.T
    bpr = nc.dram_tensor("bpr", [O, 1], f32, kind="ExternalInput")        # bp_h + pi
    csdt = nc.dram_tensor("csdt", [P, 1], f32, kind="ExternalInput")      # DT*cs
    dtom = nc.dram_tensor("dtom", [P, NT * O], f32, kind="ExternalInput")  # DT*om tiled
    wob = nc.dram_tensor("wob", [H * O + 1, D], bf16, kind="ExternalInput")  # [Wo.T; bo]
    outp = nc.dram_tensor("out", [N, D], f32, kind="ExternalOutput")

    groups = [[0, 1, 2, 3], [4, 5, 6, 7]]

    with tile.TileContext(nc) as tc:
        with (
            tc.tile_pool(name="const", bufs=1) as const,
            tc.tile_pool(name="data", bufs=1) as data,
            tc.tile_pool(name="work", bufs=2) as work,
            tc.tile_pool(name="ps2", bufs=2, space="PSUM") as ps2,
            tc.tile_pool(name="ps1", bufs=1, space="PSUM") as ps1,
            tc.tile_pool(name="dram", bufs=1, space="DRAM") as dram,
        ):
            # ---------- load inputs ----------
            xtb = data.tile([P, 2 * N], bf16)       # x.T, kt-major
            for kt in range(2):
                nc.sync.dma_start(xtb[:, kt * N:(kt + 1) * N], xT[kt * P:(kt + 1) * P, :])

            wq_s = const.tile([P, 2 * DK], bf16)
            wk_s = const.tile([P, 2 * DK], bf16)
            wp_s = const.tile([P, 2 * O], bf16)
            for kt in range(2):
                nc.sync.dma_start(wq_s[:, kt * DK:(kt + 1) * DK], wqT[kt * P:(kt + 1) * P, :])
                nc.sync.dma_start(wk_s[:, kt * DK:(kt + 1) * DK], wkT[kt * P:(kt + 1) * P, :])
                nc.sync.dma_start(wp_s[:, kt * O:(kt + 1) * O], wpT[kt * P:(kt + 1) * P, :])
            bpr_s = const.tile([O, 1], f32)
            nc.sync.dma_start(bpr_s[:, :], bpr[:, :])
            csdt_s = const.tile([P, 1], f32)
            nc.sync.dma_start(csdt_s[:, :], csdt[:, :])
            dtom_s = const.tile([P, NT * O], f32)
            nc.sync.dma_start(dtom_s[:, :], dtom[:, :])
            wob_s = const.tile([H * O + 1, D], bf16)
            nc.sync.dma_start(wob_s[:, :], wob[:, :])

            ident = const.tile([P, P], f32)
            make_identity(nc, ident[:, :])
            b_mpi = const.tile([P, 1], f32)
            nc.vector.memset(b_mpi[:, :], -PI)
            b_hpi = const.tile([P, 1], f32)
            nc.vector.memset(b_hpi[:, :], PI / 2)

            # ---------- collective warmup (absorbs first-call cost) ----------
            with tc.high_priority():
                agw_sb = const.tile([1, 8], f32)
                nc.vector.memset(agw_sb[:, :], 0.0)
                agw_in = dram.tile([1, 8], f32)
                agw_out = dram.tile([4, 8], f32)
                nc.gpsimd.dma_start(agw_in[:, :], agw_sb[:, :])
                nc.gpsimd.collective_compute(
                    "AllGather",
                    ALU.bypass,
                    replica_groups=groups,
                    ins=[agw_in[:, :].opt()],
                    outs=[agw_out[:, :].opt()],
                )

            # ---------- q/k projections (bf16) ----------
            qt = data.tile([DK, N], bf16)
            ktt = data.tile([DK, N], bf16)
            for dst, w_s in ((qt, wq_s), (ktt, wk_s)):
                for ib in range(2):
                    pq = ps2.tile([DK, 512], f32, tag="pc")
                    for kt in range(2):
                        nc.tensor.matmul(
                            pq[:, :],
                            lhsT=w_s[:, kt * DK:(kt + 1) * DK],
                            rhs=xtb[:, kt * N + ib * 512: kt * N + (ib + 1) * 512],
                            start=(kt == 0),
                            stop=(kt == 1),
                        )
                    nc.vector.tensor_copy(dst[:, ib * 512:(ib + 1) * 512], pq[:, :])

            # ---------- initial phases: phT [o, i] -> transpose to natural ----------
            phtp = ps2.tile([O, N], f32, tag="big")
            for ib in range(2):
                for kt in range(2):
                    nc.tensor.matmul(
                        phtp[:, ib * 512:(ib + 1) * 512],
                        lhsT=wp_s[:, kt * O:(kt + 1) * O],
                        rhs=xtb[:, kt * N + ib * 512: kt * N + (ib + 1) * 512],
                        start=(kt == 0),
                        stop=(kt == 1),
                    )
            pht_sb = work.tile([O, N], f32, tag="sgt")
            # + (bp + pi) while evacuating
            nc.vector.tensor_scalar(pht_sb[:, :], phtp[:, :], bpr_s[:, :], None, ALU.add)
            php = ps1.tile([P, NT * O], f32, tag="pt", bufs=2)
            for it in range(NT):
                nc.tensor.transpose(
                    php[:, it * O:(it + 1) * O],
                    pht_sb[:, it * P:(it + 1) * P],
                    ident[0:O, 0:O],
                )
            # shifted representation: ph' = wrap(ph + pi) into [0, 2pi).
            # HW tensor ops have no mod; use compare-and-correct (single
            # wrap is enough: |ph0| < 2pi and per-step drift < 0.15).
            ph = data.tile([P, NT * O], f32)
            wge = work.tile([P, NT * O], f32, tag="wge")
            wlt = work.tile([P, NT * O], f32, tag="wlt")

            def wrap_inplace(t):
                nc.vector.tensor_scalar(wge[:, :], t[:, :], TWO_PI, None, ALU.is_ge)
                nc.vector.tensor_scalar(wlt[:, :], t[:, :], 0.0, None, ALU.is_lt)
                nc.vector.tensor_tensor(wge[:, :], wlt[:, :], wge[:, :], ALU.subtract)
                nc.vector.scalar_tensor_tensor(
                    t[:, :], wge[:, :], TWO_PI, t[:, :], ALU.mult, ALU.add)

            nc.vector.tensor_copy(ph[:, :], php[:, :])
            wrap_inplace(ph)

            # ---------- scores + exp -> E^T (bf16, [j_p, jt-major i]) ----------
            etb = data.tile([P, NT * N], bf16)
            exp_insts = []
            for jt in range(NT):
                psc = ps2.tile([P, N], f32, tag="big")
                for ib in range(2):
                    nc.tensor.matmul(
                        psc[:, ib * 512:(ib + 1) * 512],
                        lhsT=ktt[:, jt * P:(jt + 1) * P],
                        rhs=qt[:, ib * 512:(ib + 1) * 512],
                        start=True,
                        stop=True,
                    )
                e_i = nc.scalar.activation(etb[:, jt * N:(jt + 1) * N], psc[:, :],
                                           ACT.Exp, scale=1.0 / math.sqrt(DK))
                exp_insts.append(e_i)

            # ---------- stationary sin/cos/ones tiles (double-buffered) ----------
            scw_a = data.tile([P, NT * SW], bf16)
            scw_b = data.tile([P, NT * SW], bf16)
            scws = [scw_a, scw_b]
            scw3s = [t[:, :].rearrange("p (t w) -> p t w", w=SW) for t in scws]
            for t in scws:
                for jt in range(NT):
                    nc.vector.memset(t[:, jt * SW + 2 * O: (jt + 1) * SW], 1.0)
            s_view = scw3s[0][:, :, 0:O]
            c_view = scw3s[0][:, :, O:2 * O]

            ph3 = ph[:, :].rearrange("p (t o) -> p t o", o=O)
            tmp = work.tile([P, NT * O], f32, tag="tmp")
            tmp3 = tmp[:, :].rearrange("p (t o) -> p t o", o=O)

            # s = sin(ph'-pi); c = cos(ph'-pi) = sin(pi/2 - |ph'-pi|)
            nc.scalar.activation(s_view, ph3, ACT.Sin, bias=b_mpi[:, :], scale=1.0)
            nc.scalar.activation(tmp3, ph3, ACT.Abs, bias=b_mpi[:, :], scale=1.0)
            ci = nc.scalar.activation(c_view, tmp3, ACT.Sin, bias=b_hpi[:, :], scale=-1.0)
            # ACT stream grouped by table set: [init sins] -> [exps] -> [step sins]
            add_dep_helper(exp_insts[0].ins, ci.ins, sync=False,
                           reason="group ACT ops by table set")

            # ---------- Kuramoto steps ----------
            gfull = data.tile([P, NT * O], f32)
            gfull3 = gfull[:, :].rearrange("p (t o) -> p t o", o=O)
            rinv = data.tile([P, NT], f32)
            dtom3 = dtom_s[:, :].rearrange("p (t o) -> p t o", o=O)
            HB = NT // 2  # it-tiles per half
            cnat = data.tile([P, NT * O], bf16)
            cnat3 = cnat[:, :].rearrange("p (t o) -> p t o", o=O)

            for step in range(STEPS):
                scw = scws[step % 2]
                scw3 = scw3s[step % 2]
                scw3_nxt = scw3s[(step + 1) % 2]
                pt = ps1.tile([P, NT * SW], f32, tag="pt", bufs=2)
                pt3 = pt[:, 0:NT * SW].rearrange("p (t w) -> p t w", w=SW)

                # --- matmuls for both halves; ib0's transposes woven into
                # ib1's MM stream so the PE never stalls on the evacuation ---
                pcs = []
                ces = []
                for ib in range(2):
                    pc = ps2.tile([SW, 512], f32, tag="pc")
                    pcs.append(pc)
                    ces.append(work.tile([SW, 512], f32, tag=f"ce{ib}", name=f"ce{ib}"))
                for jt in range(NT):
                    nc.tensor.matmul(
                        pcs[0][:, :],
                        lhsT=scw[:, jt * SW:(jt + 1) * SW],
                        rhs=etb[:, jt * N: jt * N + 512],
                        start=(jt == 0),
                        stop=(jt == NT - 1),
                    )
                nc.vector.tensor_copy(ces[0][:, :], pcs[0][:, :])
                for jt in range(NT):
                    nc.tensor.matmul(
                        pcs[1][:, :],
                        lhsT=scw[:, jt * SW:(jt + 1) * SW],
                        rhs=etb[:, jt * N + 512: jt * N + 1024],
                        start=(jt == 0),
                        stop=(jt == NT - 1),
                    )
                    if jt == 3:
                        for itl in range(HB):
                            nc.tensor.transpose(
                                pt[:, itl * SW:(itl + 1) * SW],
                                ces[0][:, itl * P:(itl + 1) * P],
                                ident[0:SW, 0:SW],
                            )
                nc.vector.tensor_copy(ces[1][:, :], pcs[1][:, :])
                for itl in range(HB):
                    it = HB + itl
                    nc.tensor.transpose(
                        pt[:, it * SW:(it + 1) * SW],
                        ces[1][:, itl * P:(itl + 1) * P],
                        ident[0:SW, 0:SW],
                    )

                # --- per-half phase updates (half 0 overlaps half-1 MMs) ---
                for hb in range(2):
                    hs = slice(hb * HB, (hb + 1) * HB)
                    es_v = pt3[:, hs, 0:O]
                    ec_v = pt3[:, hs, O:2 * O]
                    sv = scw3[:, hs, 0:O]
                    cv = scw3[:, hs, O:2 * O]
                    ph_h = ph3[:, hs, :]
                    if step == 0:
                        nc.vector.reciprocal(rinv[:, hs, None], pt3[:, hs, 2 * O:SW])
                        nc.vector.tensor_scalar(
                            gfull3[:, hs, :],
                            rinv[:, hs, None].to_broadcast((P, HB, O)),
                            csdt_s[:, :], None, ALU.mult,
                        )
                    t1 = work.tile([P, HB * O], f32, tag="t1")
                    t13 = t1[:, :].rearrange("p (t o) -> p t o", o=O)
                    t2 = work.tile([P, HB * O], f32, tag="t2")
                    t23 = t2[:, :].rearrange("p (t o) -> p t o", o=O)
                    nc.vector.tensor_tensor(t13, cv, es_v, ALU.mult)
                    nc.vector.tensor_tensor(t23, sv, ec_v, ALU.mult)
                    nc.vector.tensor_tensor(t13, t13, t23, ALU.subtract)
                    nc.vector.tensor_tensor(t13, t13, gfull3[:, hs, :], ALU.mult)
                    nc.vector.tensor_tensor(t13, t13, dtom3[:, hs, :], ALU.add)
                    nc.vector.tensor_tensor(ph_h, ph_h, t13, ALU.add)
                    # wrap into [0, 2pi): ph += 2pi*([ph<0] - [ph>=2pi])
                    nc.vector.tensor_scalar(t23, ph_h, TWO_PI, None, ALU.is_ge)
                    nc.vector.scalar_tensor_tensor(
                        t23, ph_h, 0.0, t23, ALU.is_lt, ALU.subtract)
                    nc.vector.scalar_tensor_tensor(
                        ph_h, t23, TWO_PI, ph_h, ALU.mult, ALU.add)
                    if step < STEPS - 1:
                        sv_n = scw3_nxt[:, hs, 0:O]
                        cv_n = scw3_nxt[:, hs, O:2 * O]
                        nc.scalar.activation(sv_n, ph_h, ACT.Sin, bias=b_mpi[:, :], scale=1.0)
                        nc.scalar.activation(tmp3[:, hs, :], ph_h, ACT.Abs,
                                             bias=b_mpi[:, :], scale=1.0)
                        nc.scalar.activation(cv_n, tmp3[:, hs, :], ACT.Sin,
                                             bias=b_hpi[:, :], scale=-1.0)
                    else:
                        # final sig = cos(phases), per half (starts AG sooner)
                        nc.scalar.activation(tmp3[:, hs, :], ph_h, ACT.Abs,
                                             bias=b_mpi[:, :], scale=1.0)
                        nc.scalar.activation(cnat3[:, hs, :], tmp3[:, hs, :], ACT.Sin,
                                             bias=b_hpi[:, :], scale=-1.0)

            # ---------- sig^T -> AllGather ----------
            identb = const.tile([P, P], bf16)
            nc.vector.tensor_copy(identb[:, :], ident[:, :])
            pst = ps2.tile([O, N], bf16, tag="big")
            for it in range(NT):
                nc.tensor.transpose(
                    pst[:, it * P:(it + 1) * P],
                    cnat[:, it * O:(it + 1) * O],
                    identb[:, :],
                )
            sgt = work.tile([O, N], bf16, tag="sgt2")
            nc.vector.tensor_copy(sgt[:, :], pst[:, :])

            ag_in = dram.tile([O, N], bf16)
            ag_out = dram.tile([H * O, N], bf16)
            nc.sync.dma_start(ag_in[:, :], sgt[:, :])
            nc.gpsimd.collective_compute(
                "AllGather",
                ALU.bypass,
                replica_groups=groups,
                ins=[ag_in[:, :].opt()],
                outs=[ag_out[:, :].opt()],
            )
            sgf = data.tile([H * O + 1, N], bf16)
            nc.sync.dma_start(sgf[0:H * O, :], ag_out[:, :])
            nc.vector.memset(sgf[H * O:H * O + 1, :], 1.0)

            # ---------- output projection ----------
            for it in range(NT):
                po = ps2.tile([P, D], f32, tag="pc")
                nc.tensor.matmul(po[:, :], lhsT=sgf[:, it * P:(it + 1) * P],
                                 rhs=wob_s[:, :], start=True, stop=True)
                ot = work.tile([P, D], f32, tag="ot")
                nc.vector.tensor_copy(ot[:, :], po[:, :])
                nc.sync.dma_start(outp[it * P:(it + 1) * P, :], ot[:, :])

    nc.compile()
    return nc


def get_nc():
    if "nc" not in _CACHE:
        _CACHE["nc"] = _build_nc()
    return _CACHE["nc"]


def make_in_maps(x, Wq, Wk, Wp, bp, Wo, bo, omega, coupling_scale):
    import concourse.mybir as mybir

    bf16 = mybir.dt.np(mybir.dt.bfloat16)
    x = np.asarray(x, np.float32)
    Wq = np.asarray(Wq, np.float32)
    Wk = np.asarray(Wk, np.float32)
    Wp = np.asarray(Wp, np.float32)
    bp = np.asarray(bp, np.float32)
    Wo = np.asarray(Wo, np.float32)
    bo = np.asarray(bo, np.float32)
    omega = np.asarray(omega, np.float32)
    cs = float(np.asarray(coupling_scale, np.float32))

    wob_full = np.ascontiguousarray(
        np.concatenate([Wo.T, bo[None, :]], axis=0)).astype(bf16)
    csdt_full = np.full((P, 1), DT * cs, np.float32)

    in_maps = []
    for c in range(NCORES):
        b, h = c // H, c % H
        in_maps.append({
            "xT": np.ascontiguousarray(x[b].T).astype(bf16),
            "wqT": np.ascontiguousarray(Wq[h * DK:(h + 1) * DK, :].T).astype(bf16),
            "wkT": np.ascontiguousarray(Wk[h * DK:(h + 1) * DK, :].T).astype(bf16),
            "wpT": np.ascontiguousarray(Wp[h * O:(h + 1) * O, :].T).astype(bf16),
            "bpr": np.ascontiguousarray(
                (bp[h * O:(h + 1) * O] + np.pi)[:, None], np.float32),
            "csdt": csdt_full,
            "dtom": np.ascontiguousarray(
                np.tile((DT * omega[h])[None, :], (P, NT)), np.float32),
            "wob": wob_full,
        })
    return in_maps


def run_on_hw(in_maps, trace=False):
    from concourse.bass_utils import run_bass_kernel_spmd

    nc = get_nc()
    return run_bass_kernel_spmd(nc, in_maps, core_ids=list(range(NCORES)), trace=trace)


def kernel(x, Wq, Wk, Wp, bp, Wo, bo, omega, coupling_scale):
    in_maps = make_in_maps(x, Wq, Wk, Wp, bp, Wo, bo, omega, coupling_scale)
    res = run_on_hw(in_maps, trace=False)
    out = np.stack([res.results[0]["out"], res.results[H]["out"]], axis=0)
    return np.ascontiguousarray(out, np.float32)


# revision 18
# speedup vs baseline: 1.3024x; 1.2403x over previous
"""AKOrN layer (attention-coupled Kuramoto oscillators) on 8 TRN2 NeuronCores.

Sharding: B*H = 2*4 = 8 (batch, head) pairs -> one pair per core.
Each core computes its head's attention matrix E = exp(scores) entirely in
SBUF (never touches HBM), runs the 5 Kuramoto steps locally, then the four
cores of each batch AllGather their cos(phases) (32KB) and every core computes
the full output projection for its batch. Host picks core 0 -> batch 0,
core 4 -> batch 1.

Self-contained: hardcodes all shapes; only imports concourse from the
container's /opt/trn_rl_repo.
"""

import math
import sys

import numpy as np

for _p in ("/opt/trn_rl_repo",):
    if _p not in sys.path:
        sys.path.insert(0, _p)

# Problem constants (from the reference nn.Module)
B, N, D, H, O = 2, 1024, 256, 4, 8
DT, STEPS = 0.1, 5
DK = D // H            # 64 head dim
P = 128                # partitions
NT = N // P            # 8 token tiles
NCORES = 8
SW = 2 * O + 1         # stationary width per j-tile: [sin | cos | ones] = 17
PI = float(np.pi)
TWO_PI = float(2 * np.pi)

_CACHE = {}


def _build_nc():
    import concourse.bacc as bacc
    import concourse.tile as tile
    import concourse.mybir as mybir
    from concourse.masks import make_identity
    from concourse.tile_rust import add_dep_helper

    f32 = mybir.dt.float32
    bf16 = mybir.dt.bfloat16
    ALU = mybir.AluOpType
    ACT = mybir.ActivationFunctionType

    nc = bacc.Bacc(
        "TRN2",
        target_bir_lowering=False,
        debug=False,
        enable_asserts=False,
        num_devices=NCORES,
    )

    # Per-core external inputs (host pre-sliced / transposed)
    xT = nc.dram_tensor("xT", [D, N], bf16, kind="ExternalInput")         # x[b].T (bf16)
    wqT = nc.dram_tensor("wqT", [D, DK], bf16, kind="ExternalInput")      # Wq_h.T
    wkT = nc.dram_tensor("wkT", [D, DK], bf16, kind="ExternalInput")      # Wk_h.T
    wpT = nc.dram_tensor("wpT", [D, O], bf16, kind="ExternalInput")       # Wp_h.T
    bpr = nc.dram_tensor("bpr", [O, 1], f32, kind="ExternalInput")        # bp_h + pi
    csdt = nc.dram_tensor("csdt", [P, 1], f32, kind="ExternalInput")      # DT*cs
    dtom = nc.dram_tensor("dtom", [P, NT * O], f32, kind="ExternalInput")  # DT*om tiled
    wob = nc.dram_tensor("wob", [H * O + 1, D], bf16, kind="ExternalInput")  # [Wo.T; bo]
    outp = nc.dram_tensor("out", [N, D], f32, kind="ExternalOutput")

    groups = [[0, 1, 2, 3], [4, 5, 6, 7]]

    with tile.TileContext(nc) as tc:
        with (
            tc.tile_pool(name="const", bufs=1) as const,
            tc.tile_pool(name="data", bufs=1) as data,
            tc.tile_pool(name="work", bufs=2) as work,
            tc.tile_pool(name="ps2", bufs=2, space="PSUM") as ps2,
            tc.tile_pool(name="ps1", bufs=1, space="PSUM") as ps1,
            tc.tile_pool(name="dram", bufs=1, space="DRAM") as dram,
        ):
            # ---------- load inputs ----------
            xtb = data.tile([P, 2 * N], bf16)       # x.T, kt-major
            for kt in range(2):
                nc.sync.dma_start(xtb[:, kt * N:(kt + 1) * N], xT[kt * P:(kt + 1) * P, :])

            wq_s = const.tile([P, 2 * DK], bf16)
            wk_s = const.tile([P, 2 * DK], bf16)
            wp_s = const.tile([P, 2 * O], bf16)
            for kt in range(2):
                nc.sync.dma_start(wq_s[:, kt * DK:(kt + 1) * DK], wqT[kt * P:(kt + 1) * P, :])
                nc.sync.dma_start(wk_s[:, kt * DK:(kt + 1) * DK], wkT[kt * P:(kt + 1) * P, :])
                nc.sync.dma_start(wp_s[:, kt * O:(kt + 1) * O], wpT[kt * P:(kt + 1) * P, :])
            bpr_s = const.tile([O, 1], f32)
            nc.sync.dma_start(bpr_s[:, :], bpr[:, :])
            csdt_s = const.tile([P, 1], f32)
            nc.sync.dma_start(csdt_s[:, :], csdt[:, :])
            dtom_s = const.tile([P, NT * O], f32)
            nc.sync.dma_start(dtom_s[:, :], dtom[:, :])
            wob_s = const.tile([H * O + 1, D], bf16)
            nc.sync.dma_start(wob_s[:, :], wob[:, :])

            ident = const.tile([P, P], f32)
            make_identity(nc, ident[:, :])
            b_mpi = const.tile([P, 1], f32)
            nc.vector.memset(b_mpi[:, :], -PI)
            b_hpi = const.tile([P, 1], f32)
            nc.vector.memset(b_hpi[:, :], PI / 2)

            # ---------- collective warmup (absorbs first-call cost) ----------
            with tc.high_priority():
                agw_sb = const.tile([1, 8], f32)
                nc.vector.memset(agw_sb[:, :], 0.0)
                agw_in = dram.tile([1, 8], f32)
                agw_out = dram.tile([4, 8], f32)
                nc.gpsimd.dma_start(agw_in[:, :], agw_sb[:, :])
                nc.gpsimd.collective_compute(
                    "AllGather",
                    ALU.bypass,
                    replica_groups=groups,
                    ins=[agw_in[:, :].opt()],
                    outs=[agw_out[:, :].opt()],
                )

            # ---------- q/k projections (bf16) ----------
            qt = data.tile([DK, N], bf16)
            ktt = data.tile([DK, N], bf16)
            for dst, w_s in ((qt, wq_s), (ktt, wk_s)):
                for ib in range(2):
                    pq = ps2.tile([DK, 512], f32, tag="pc")
                    for kt in range(2):
                        nc.tensor.matmul(
                            pq[:, :],
                            lhsT=w_s[:, kt * DK:(kt + 1) * DK],
                            rhs=xtb[:, kt * N + ib * 512: kt * N + (ib + 1) * 512],
                            start=(kt == 0),
                            stop=(kt == 1),
                        )
                    nc.vector.tensor_copy(dst[:, ib * 512:(ib + 1) * 512], pq[:, :])

            # ---------- initial phases: phT [o, i] -> transpose to natural ----------
            phtp = ps2.tile([O, N], f32, tag="big")
            for ib in range(2):
                for kt in range(2):
                    nc.tensor.matmul(
                        phtp[:, ib * 512:(ib + 1) * 512],
                        lhsT=wp_s[:, kt * O:(kt + 1) * O],
                        rhs=xtb[:, kt * N + ib * 512: kt * N + (ib + 1) * 512],
                        start=(kt == 0),
                        stop=(kt == 1),
                    )
            pht_sb = work.tile([O, N], f32, tag="sgt")
            # + (bp + pi) while evacuating
            nc.vector.tensor_scalar(pht_sb[:, :], phtp[:, :], bpr_s[:, :], None, ALU.add)
            php = ps1.tile([P, NT * O], f32, tag="pt", bufs=2)
            for it in range(NT):
                nc.tensor.transpose(
                    php[:, it * O:(it + 1) * O],
                    pht_sb[:, it * P:(it + 1) * P],
                    ident[0:O, 0:O],
                )
            # shifted representation: ph' = wrap(ph + pi) into [0, 2pi).
            # HW tensor ops have no mod; use compare-and-correct (single
            # wrap is enough: |ph0| < 2pi and per-step drift < 0.15).
            ph = data.tile([P, NT * O], f32)
            wge = work.tile([P, NT * O], f32, tag="wge")
            wlt = work.tile([P, NT * O], f32, tag="wlt")

            def wrap_inplace(t):
                nc.vector.tensor_scalar(wge[:, :], t[:, :], TWO_PI, None, ALU.is_ge)
                nc.vector.tensor_scalar(wlt[:, :], t[:, :], 0.0, None, ALU.is_lt)
                nc.vector.tensor_tensor(wge[:, :], wlt[:, :], wge[:, :], ALU.subtract)
                nc.vector.scalar_tensor_tensor(
                    t[:, :], wge[:, :], TWO_PI, t[:, :], ALU.mult, ALU.add)

            nc.vector.tensor_copy(ph[:, :], php[:, :])
            wrap_inplace(ph)

            # ---------- scores + exp -> E^T (bf16, [j_p, jt-major i]) ----------
            etb = data.tile([P, NT * N], bf16)
            exp_insts = []
            for jt in range(NT):
                psc = ps2.tile([P, N], f32, tag="big")
                for ib in range(2):
                    nc.tensor.matmul(
                        psc[:, ib * 512:(ib + 1) * 512],
                        lhsT=ktt[:, jt * P:(jt + 1) * P],
                        rhs=qt[:, ib * 512:(ib + 1) * 512],
                        start=True,
                        stop=True,
                    )
                e_i = nc.scalar.activation(etb[:, jt * N:(jt + 1) * N], psc[:, :],
                                           ACT.Exp, scale=1.0 / math.sqrt(DK))
                exp_insts.append(e_i)

            # ---------- stationary sin/cos/ones tiles (double-buffered) ----------
            scw_a = data.tile([P, NT * SW], bf16)
            scw_b = data.tile([P, NT * SW], bf16)
            scws = [scw_a, scw_b]
            scw3s = [t[:, :].rearrange("p (t w) -> p t w", w=SW) for t in scws]
            for t in scws:
                for jt in range(NT):
                    nc.vector.memset(t[:, jt * SW + 2 * O: (jt + 1) * SW], 1.0)
            s_view = scw3s[0][:, :, 0:O]
            c_view = scw3s[0][:, :, O:2 * O]

            ph3 = ph[:, :].rearrange("p (t o) -> p t o", o=O)
            tmp = work.tile([P, NT * O], f32, tag="tmp")
            tmp3 = tmp[:, :].rearrange("p (t o) -> p t o", o=O)

            # s = sin(ph'-pi); c = cos(ph'-pi) = sin(pi/2 - |ph'-pi|)
            nc.scalar.activation(s_view, ph3, ACT.Sin, bias=b_mpi[:, :], scale=1.0)
            nc.scalar.activation(tmp3, ph3, ACT.Abs, bias=b_mpi[:, :], scale=1.0)
            ci = nc.scalar.activation(c_view, tmp3, ACT.Sin, bias=b_hpi[:, :], scale=-1.0)
            # ACT stream grouped by table set: [init sins] -> [exps] -> [step sins]
            for k, e_i in enumerate(exp_insts):
                add_dep_helper(e_i.ins, ci.ins, sync=(k == 0),
                               reason="group ACT ops by table set")

            # ---------- Kuramoto steps ----------
            gfull = data.tile([P, NT * O], f32)
            gfull3 = gfull[:, :].rearrange("p (t o) -> p t o", o=O)
            rinv = data.tile([P, NT], f32)
            dtom3 = dtom_s[:, :].rearrange("p (t o) -> p t o", o=O)
            HB = NT // 2  # it-tiles per half
            cnat = data.tile([P, NT * O], bf16)
            cnat3 = cnat[:, :].rearrange("p (t o) -> p t o", o=O)

            def half_update(step, hb, pt3, scw3, scw3_nxt):
                hs = slice(hb * HB, (hb + 1) * HB)
                es_v = pt3[:, hs, 0:O]
                ec_v = pt3[:, hs, O:2 * O]
                sv = scw3[:, hs, 0:O]
                cv = scw3[:, hs, O:2 * O]
                ph_h = ph3[:, hs, :]
                if step == 0:
                    nc.vector.reciprocal(rinv[:, hs, None], pt3[:, hs, 2 * O:SW])
                    nc.vector.tensor_scalar(
                        gfull3[:, hs, :],
                        rinv[:, hs, None].to_broadcast((P, HB, O)),
                        csdt_s[:, :], None, ALU.mult,
                    )
                t1 = work.tile([P, HB * O], f32, tag="t1", name="t1")
                t13 = t1[:, :].rearrange("p (t o) -> p t o", o=O)
                t2 = work.tile([P, HB * O], f32, tag="t2", name="t2")
                t23 = t2[:, :].rearrange("p (t o) -> p t o", o=O)
                nc.vector.tensor_tensor(t13, cv, es_v, ALU.mult)
                nc.vector.tensor_tensor(t23, sv, ec_v, ALU.mult)
                nc.vector.tensor_tensor(t13, t13, t23, ALU.subtract)
                nc.vector.tensor_tensor(t13, t13, gfull3[:, hs, :], ALU.mult)
                nc.vector.tensor_tensor(t13, t13, dtom3[:, hs, :], ALU.add)
                nc.vector.tensor_tensor(ph_h, ph_h, t13, ALU.add)
                # wrap into [0, 2pi): ph += 2pi*([ph<0] - [ph>=2pi])
                nc.vector.tensor_scalar(t23, ph_h, TWO_PI, None, ALU.is_ge)
                nc.vector.scalar_tensor_tensor(
                    t23, ph_h, 0.0, t23, ALU.is_lt, ALU.subtract)
                nc.vector.scalar_tensor_tensor(
                    ph_h, t23, TWO_PI, ph_h, ALU.mult, ALU.add)
                if step < STEPS - 1:
                    sv_n = scw3_nxt[:, hs, 0:O]
                    cv_n = scw3_nxt[:, hs, O:2 * O]
                    nc.scalar.activation(sv_n, ph_h, ACT.Sin, bias=b_mpi[:, :], scale=1.0)
                    nc.scalar.activation(tmp3[:, hs, :], ph_h, ACT.Abs,
                                         bias=b_mpi[:, :], scale=1.0)
                    nc.scalar.activation(cv_n, tmp3[:, hs, :], ACT.Sin,
                                         bias=b_hpi[:, :], scale=-1.0)
                else:
                    # final sig = cos(phases), per half (starts AG sooner)
                    nc.scalar.activation(tmp3[:, hs, :], ph_h, ACT.Abs,
                                         bias=b_mpi[:, :], scale=1.0)
                    nc.scalar.activation(cnat3[:, hs, :], tmp3[:, hs, :], ACT.Sin,
                                         bias=b_hpi[:, :], scale=-1.0)

            for step in range(STEPS):
                scw = scws[step % 2]
                scw3 = scw3s[step % 2]
                scw3_nxt = scw3s[(step + 1) % 2]
                pt = ps1.tile([P, NT * SW], f32, tag="pt", bufs=2)
                pt3 = pt[:, 0:NT * SW].rearrange("p (t w) -> p t w", w=SW)

                # ib0 matmuls
                pc0 = ps2.tile([SW, 512], f32, tag="pc", name="pc0")
                for jt in range(NT):
                    nc.tensor.matmul(
                        pc0[:, :],
                        lhsT=scw[:, jt * SW:(jt + 1) * SW],
                        rhs=etb[:, jt * N: jt * N + 512],
                        start=(jt == 0),
                        stop=(jt == NT - 1),
                    )
                ce0 = work.tile([SW, 512], f32, tag="ce0", name="ce0")
                nc.vector.tensor_copy(ce0[:, :], pc0[:, :])
                # ib1 matmuls with ib0's transposes + update woven in: the
                # PE reaches the transposes only after ce0 is long ready, and
                # the DVE/ACT half-0 update overlaps the rest of ib1's MMs
                pc1 = ps2.tile([SW, 512], f32, tag="pc", name="pc1")
                for jt in range(NT):
                    nc.tensor.matmul(
                        pc1[:, :],
                        lhsT=scw[:, jt * SW:(jt + 1) * SW],
                        rhs=etb[:, jt * N + 512: jt * N + 1024],
                        start=(jt == 0),
                        stop=(jt == NT - 1),
                    )
                    if jt == 2:
                        for itl in range(HB):
                            nc.tensor.transpose(
                                pt[:, itl * SW:(itl + 1) * SW],
                                ce0[:, itl * P:(itl + 1) * P],
                                ident[0:SW, 0:SW],
                            )
                    if jt == 3:
                        half_update(step, 0, pt3, scw3, scw3_nxt)
                ce1 = work.tile([SW, 512], f32, tag="ce1", name="ce1")
                nc.vector.tensor_copy(ce1[:, :], pc1[:, :])
                for itl in range(HB):
                    it = HB + itl
                    nc.tensor.transpose(
                        pt[:, it * SW:(it + 1) * SW],
                        ce1[:, itl * P:(itl + 1) * P],
                        ident[0:SW, 0:SW],
                    )
                half_update(step, 1, pt3, scw3, scw3_nxt)

            # ---------- sig^T -> AllGather ----------
            identb = const.tile([P, P], bf16)
            nc.vector.tensor_copy(identb[:, :], ident[:, :])
            pst = ps2.tile([O, N], bf16, tag="big")
            for it in range(NT):
                nc.tensor.transpose(
                    pst[:, it * P:(it + 1) * P],
                    cnat[:, it * O:(it + 1) * O],
                    identb[:, :],
                )
            sgt = work.tile([O, N], bf16, tag="sgt2")
            nc.vector.tensor_copy(sgt[:, :], pst[:, :])

            ag_in = dram.tile([O, N], bf16)
            ag_out = dram.tile([H * O, N], bf16)
            nc.sync.dma_start(ag_in[:, :], sgt[:, :])
            nc.gpsimd.collective_compute(
                "AllGather",
                ALU.bypass,
                replica_groups=groups,
                ins=[ag_in[:, :].opt()],
                outs=[ag_out[:, :].opt()],
            )
            sgf = data.tile([H * O + 1, N], bf16)
            nc.sync.dma_start(sgf[0:H * O, :], ag_out[:, :])
            nc.vector.memset(sgf[H * O:H * O + 1, :], 1.0)

            # ---------- output projection ----------
            for it in range(NT):
                po = ps2.tile([P, D], f32, tag="pc")
                nc.tensor.matmul(po[:, :], lhsT=sgf[:, it * P:(it + 1) * P],
                                 rhs=wob_s[:, :], start=True, stop=True)
                ot = work.tile([P, D], f32, tag="ot")
                nc.vector.tensor_copy(ot[:, :], po[:, :])
                nc.sync.dma_start(outp[it * P:(it + 1) * P, :], ot[:, :])

    nc.compile()
    return nc


def get_nc():
    if "nc" not in _CACHE:
        _CACHE["nc"] = _build_nc()
    return _CACHE["nc"]


def make_in_maps(x, Wq, Wk, Wp, bp, Wo, bo, omega, coupling_scale):
    import concourse.mybir as mybir

    bf16 = mybir.dt.np(mybir.dt.bfloat16)
    x = np.asarray(x, np.float32)
    Wq = np.asarray(Wq, np.float32)
    Wk = np.asarray(Wk, np.float32)
    Wp = np.asarray(Wp, np.float32)
    bp = np.asarray(bp, np.float32)
    Wo = np.asarray(Wo, np.float32)
    bo = np.asarray(bo, np.float32)
    omega = np.asarray(omega, np.float32)
    cs = float(np.asarray(coupling_scale, np.float32))

    wob_full = np.ascontiguousarray(
        np.concatenate([Wo.T, bo[None, :]], axis=0)).astype(bf16)
    csdt_full = np.full((P, 1), DT * cs, np.float32)

    in_maps = []
    for c in range(NCORES):
        b, h = c // H, c % H
        in_maps.append({
            "xT": np.ascontiguousarray(x[b].T).astype(bf16),
            "wqT": np.ascontiguousarray(Wq[h * DK:(h + 1) * DK, :].T).astype(bf16),
            "wkT": np.ascontiguousarray(Wk[h * DK:(h + 1) * DK, :].T).astype(bf16),
            "wpT": np.ascontiguousarray(Wp[h * O:(h + 1) * O, :].T).astype(bf16),
            "bpr": np.ascontiguousarray(
                (bp[h * O:(h + 1) * O] + np.pi)[:, None], np.float32),
            "csdt": csdt_full,
            "dtom": np.ascontiguousarray(
                np.tile((DT * omega[h])[None, :], (P, NT)), np.float32),
            "wob": wob_full,
        })
    return in_maps


def run_on_hw(in_maps, trace=False):
    from concourse.bass_utils import run_bass_kernel_spmd

    nc = get_nc()
    return run_bass_kernel_spmd(nc, in_maps, core_ids=list(range(NCORES)), trace=trace)


def kernel(x, Wq, Wk, Wp, bp, Wo, bo, omega, coupling_scale):
    in_maps = make_in_maps(x, Wq, Wk, Wp, bp, Wo, bo, omega, coupling_scale)
    res = run_on_hw(in_maps, trace=False)
    out = np.stack([res.results[0]["out"], res.results[H]["out"]], axis=0)
    return np.ascontiguousarray(out, np.float32)


# revision 19
# speedup vs baseline: 1.4420x; 1.1072x over previous
"""AKOrN layer (attention-coupled Kuramoto oscillators) on 8 TRN2 NeuronCores.

Sharding: B*H = 2*4 = 8 (batch, head) pairs -> one pair per core.
Each core computes its head's attention matrix E = exp(scores) entirely in
SBUF (never touches HBM), runs the 5 Kuramoto steps locally, then the four
cores of each batch AllGather their cos(phases) (32KB) and every core computes
the full output projection for its batch. Host picks core 0 -> batch 0,
core 4 -> batch 1.

Self-contained: hardcodes all shapes; only imports concourse from the
container's /opt/trn_rl_repo.
"""

import math
import sys

import numpy as np

for _p in ("/opt/trn_rl_repo",):
    if _p not in sys.path:
        sys.path.insert(0, _p)

# Problem constants (from the reference nn.Module)
B, N, D, H, O = 2, 1024, 256, 4, 8
DT, STEPS = 0.1, 5
DK = D // H            # 64 head dim
P = 128                # partitions
NT = N // P            # 8 token tiles
NCORES = 8
SW = 2 * O + 1         # stationary width per j-tile: [sin | cos | ones] = 17
PI = float(np.pi)
TWO_PI = float(2 * np.pi)

_CACHE = {}


def _build_nc():
    import concourse.bacc as bacc
    import concourse.tile as tile
    import concourse.mybir as mybir
    from concourse.masks import make_identity
    from concourse.tile_rust import add_dep_helper

    f32 = mybir.dt.float32
    bf16 = mybir.dt.bfloat16
    ALU = mybir.AluOpType
    ACT = mybir.ActivationFunctionType

    nc = bacc.Bacc(
        "TRN2",
        target_bir_lowering=False,
        debug=False,
        enable_asserts=False,
        num_devices=NCORES,
    )

    # Per-core external inputs (host pre-sliced / transposed)
    xT = nc.dram_tensor("xT", [D, N], bf16, kind="ExternalInput")         # x[b].T (bf16)
    wqT = nc.dram_tensor("wqT", [D, DK], bf16, kind="ExternalInput")      # Wq_h.T
    wkT = nc.dram_tensor("wkT", [D, DK], bf16, kind="ExternalInput")      # Wk_h.T
    wpT = nc.dram_tensor("wpT", [D, O], bf16, kind="ExternalInput")       # Wp_h.T
    bpr = nc.dram_tensor("bpr", [O, 1], f32, kind="ExternalInput")        # bp_h + pi
    csdt = nc.dram_tensor("csdt", [P, 1], f32, kind="ExternalInput")      # DT*cs
    dtom = nc.dram_tensor("dtom", [P, NT * O], f32, kind="ExternalInput")  # DT*om tiled
    wob = nc.dram_tensor("wob", [H * O + 1, D], bf16, kind="ExternalInput")  # [Wo.T; bo]
    outp = nc.dram_tensor("out", [N, D], f32, kind="ExternalOutput")

    groups = [[0, 1, 2, 3], [4, 5, 6, 7]]

    with tile.TileContext(nc) as tc:
        with (
            tc.tile_pool(name="const", bufs=1) as const,
            tc.tile_pool(name="data", bufs=1) as data,
            tc.tile_pool(name="work", bufs=2) as work,
            tc.tile_pool(name="ps2", bufs=2, space="PSUM") as ps2,
            tc.tile_pool(name="ps1", bufs=1, space="PSUM") as ps1,
            tc.tile_pool(name="dram", bufs=1, space="DRAM") as dram,
        ):
            # ---------- load inputs ----------
            xtb = data.tile([P, 2 * N], bf16)       # x.T, kt-major
            for kt in range(2):
                nc.sync.dma_start(xtb[:, kt * N:(kt + 1) * N], xT[kt * P:(kt + 1) * P, :])

            wq_s = const.tile([P, 2 * DK], bf16)
            wk_s = const.tile([P, 2 * DK], bf16)
            wp_s = const.tile([P, 2 * O], bf16)
            for kt in range(2):
                nc.sync.dma_start(wq_s[:, kt * DK:(kt + 1) * DK], wqT[kt * P:(kt + 1) * P, :])
                nc.sync.dma_start(wk_s[:, kt * DK:(kt + 1) * DK], wkT[kt * P:(kt + 1) * P, :])
                nc.sync.dma_start(wp_s[:, kt * O:(kt + 1) * O], wpT[kt * P:(kt + 1) * P, :])
            bpr_s = const.tile([O, 1], f32)
            nc.sync.dma_start(bpr_s[:, :], bpr[:, :])
            csdt_s = const.tile([P, 1], f32)
            nc.sync.dma_start(csdt_s[:, :], csdt[:, :])
            dtom_s = const.tile([P, NT * O], f32)
            nc.sync.dma_start(dtom_s[:, :], dtom[:, :])
            wob_s = const.tile([H * O + 1, D], bf16)
            nc.sync.dma_start(wob_s[:, :], wob[:, :])

            ident = const.tile([P, P], f32)
            make_identity(nc, ident[:, :])
            b_mpi = const.tile([P, 1], f32)
            nc.vector.memset(b_mpi[:, :], -PI)
            b_hpi = const.tile([P, 1], f32)
            nc.vector.memset(b_hpi[:, :], PI / 2)

            # ---------- collective warmup (absorbs first-call cost) ----------
            with tc.high_priority():
                agw_sb = const.tile([1, 8], f32)
                nc.vector.memset(agw_sb[:, :], 0.0)
                agw_in = dram.tile([1, 8], f32)
                agw_out = dram.tile([4, 8], f32)
                nc.gpsimd.dma_start(agw_in[:, :], agw_sb[:, :])
                nc.gpsimd.collective_compute(
                    "AllGather",
                    ALU.bypass,
                    replica_groups=groups,
                    ins=[agw_in[:, :].opt()],
                    outs=[agw_out[:, :].opt()],
                )

            # ---------- q/k projections (bf16) ----------
            qt = data.tile([DK, N], bf16)
            ktt = data.tile([DK, N], bf16)
            for dst, w_s in ((qt, wq_s), (ktt, wk_s)):
                for ib in range(2):
                    pq = ps2.tile([DK, 512], f32, tag="pc")
                    for kt in range(2):
                        nc.tensor.matmul(
                            pq[:, :],
                            lhsT=w_s[:, kt * DK:(kt + 1) * DK],
                            rhs=xtb[:, kt * N + ib * 512: kt * N + (ib + 1) * 512],
                            start=(kt == 0),
                            stop=(kt == 1),
                        )
                    nc.vector.tensor_copy(dst[:, ib * 512:(ib + 1) * 512], pq[:, :])

            # ---------- initial phases: phT [o, i] -> transpose to natural ----------
            phtp = ps2.tile([O, N], f32, tag="big")
            for ib in range(2):
                for kt in range(2):
                    nc.tensor.matmul(
                        phtp[:, ib * 512:(ib + 1) * 512],
                        lhsT=wp_s[:, kt * O:(kt + 1) * O],
                        rhs=xtb[:, kt * N + ib * 512: kt * N + (ib + 1) * 512],
                        start=(kt == 0),
                        stop=(kt == 1),
                    )
            pht_sb = work.tile([O, N], f32, tag="sgt")
            # + (bp + pi) while evacuating
            nc.vector.tensor_scalar(pht_sb[:, :], phtp[:, :], bpr_s[:, :], None, ALU.add)
            php = ps1.tile([P, NT * O], f32, tag="pt", bufs=2)
            for it in range(NT):
                nc.tensor.transpose(
                    php[:, it * O:(it + 1) * O],
                    pht_sb[:, it * P:(it + 1) * P],
                    ident[0:O, 0:O],
                )
            # shifted representation: ph' = wrap(ph + pi) into [0, 2pi).
            # HW tensor ops have no mod; use compare-and-correct (single
            # wrap is enough: |ph0| < 2pi and per-step drift < 0.15).
            ph = data.tile([P, NT * O], f32)
            wge = work.tile([P, NT * O], f32, tag="wge")
            wlt = work.tile([P, NT * O], f32, tag="wlt")

            def wrap_inplace(t):
                nc.vector.tensor_scalar(wge[:, :], t[:, :], TWO_PI, None, ALU.is_ge)
                nc.vector.tensor_scalar(wlt[:, :], t[:, :], 0.0, None, ALU.is_lt)
                nc.vector.tensor_tensor(wge[:, :], wlt[:, :], wge[:, :], ALU.subtract)
                nc.vector.scalar_tensor_tensor(
                    t[:, :], wge[:, :], TWO_PI, t[:, :], ALU.mult, ALU.add)

            nc.vector.tensor_copy(ph[:, :], php[:, :])
            wrap_inplace(ph)

            # ---------- scores + exp -> E^T (bf16, [j_p, jt-major i]) ----------
            etb = data.tile([P, NT * N], bf16)
            exp_insts = []
            for jt in range(NT):
                psc = ps2.tile([P, N], f32, tag="big")
                for ib in range(2):
                    nc.tensor.matmul(
                        psc[:, ib * 512:(ib + 1) * 512],
                        lhsT=ktt[:, jt * P:(jt + 1) * P],
                        rhs=qt[:, ib * 512:(ib + 1) * 512],
                        start=True,
                        stop=True,
                    )
                e_i = nc.scalar.activation(etb[:, jt * N:(jt + 1) * N], psc[:, :],
                                           ACT.Exp, scale=1.0 / math.sqrt(DK))
                exp_insts.append(e_i)

            # ---------- stationary sin/cos/ones tiles ----------
            # double-buffered by step parity AND split lo/hi so next step's
            # first accumulations only depend on the lo-half sins
            HBT = NT // 2
            scw_al = data.tile([P, HBT * SW], bf16)
            scw_ah = data.tile([P, HBT * SW], bf16)
            scw_bl = data.tile([P, HBT * SW], bf16)
            scw_bh = data.tile([P, HBT * SW], bf16)
            scws = [(scw_al, scw_ah), (scw_bl, scw_bh)]
            scw3s = [tuple(t[:, :].rearrange("p (t w) -> p t w", w=SW) for t in pair)
                     for pair in scws]
            for pair in scws:
                for t in pair:
                    for jt in range(HBT):
                        nc.vector.memset(t[:, jt * SW + 2 * O: (jt + 1) * SW], 1.0)

            ph3 = ph[:, :].rearrange("p (t o) -> p t o", o=O)
            tmp = work.tile([P, NT * O], f32, tag="tmp")
            tmp3 = tmp[:, :].rearrange("p (t o) -> p t o", o=O)

            # s = sin(ph'-pi); c = cos(ph'-pi) = sin(pi/2 - |ph'-pi|)
            nc.scalar.activation(tmp3, ph3, ACT.Abs, bias=b_mpi[:, :], scale=1.0)
            ci = None
            for hb in range(2):
                hs = slice(hb * HBT, (hb + 1) * HBT)
                s3h = scw3s[0][hb][:, :, 0:O]
                c3h = scw3s[0][hb][:, :, O:2 * O]
                nc.scalar.activation(s3h, ph3[:, hs, :], ACT.Sin, bias=b_mpi[:, :], scale=1.0)
                ci = nc.scalar.activation(c3h, tmp3[:, hs, :], ACT.Sin,
                                          bias=b_hpi[:, :], scale=-1.0)
            # ACT stream grouped by table set: [init sins] -> [exps] -> [step sins]
            for k, e_i in enumerate(exp_insts):
                add_dep_helper(e_i.ins, ci.ins, sync=(k == 0),
                               reason="group ACT ops by table set")

            # ---------- Kuramoto steps ----------
            gfull = data.tile([P, NT * O], f32)
            gfull3 = gfull[:, :].rearrange("p (t o) -> p t o", o=O)
            rinv = data.tile([P, NT], f32)
            dtom3 = dtom_s[:, :].rearrange("p (t o) -> p t o", o=O)
            HB = NT // 2  # it-tiles per half
            cnat = data.tile([P, NT * O], bf16)
            cnat3 = cnat[:, :].rearrange("p (t o) -> p t o", o=O)

            def half_update(step, hb, pt3, scw3, scw3_nxt):
                hs = slice(hb * HB, (hb + 1) * HB)
                es_v = pt3[:, hs, 0:O]
                ec_v = pt3[:, hs, O:2 * O]
                sv = scw3[hb][:, :, 0:O]
                cv = scw3[hb][:, :, O:2 * O]
                ph_h = ph3[:, hs, :]
                if step == 0:
                    nc.vector.reciprocal(rinv[:, hs, None], pt3[:, hs, 2 * O:SW])
                    nc.vector.tensor_scalar(
                        gfull3[:, hs, :],
                        rinv[:, hs, None].to_broadcast((P, HB, O)),
                        csdt_s[:, :], None, ALU.mult,
                    )
                t1 = work.tile([P, HB * O], f32, tag="t1", name="t1")
                t13 = t1[:, :].rearrange("p (t o) -> p t o", o=O)
                t2 = work.tile([P, HB * O], f32, tag="t2", name="t2")
                t23 = t2[:, :].rearrange("p (t o) -> p t o", o=O)
                nc.vector.tensor_tensor(t13, cv, es_v, ALU.mult)
                nc.vector.tensor_tensor(t23, sv, ec_v, ALU.mult)
                nc.vector.tensor_tensor(t13, t13, t23, ALU.subtract)
                nc.vector.tensor_tensor(t13, t13, gfull3[:, hs, :], ALU.mult)
                nc.vector.tensor_tensor(t13, t13, dtom3[:, hs, :], ALU.add)
                nc.vector.tensor_tensor(ph_h, ph_h, t13, ALU.add)
                # wrap into [0, 2pi): ph += 2pi*([ph<0] - [ph>=2pi])
                nc.vector.tensor_scalar(t23, ph_h, TWO_PI, None, ALU.is_ge)
                nc.vector.scalar_tensor_tensor(
                    t23, ph_h, 0.0, t23, ALU.is_lt, ALU.subtract)
                nc.vector.scalar_tensor_tensor(
                    ph_h, t23, TWO_PI, ph_h, ALU.mult, ALU.add)
                if step < STEPS - 1:
                    sv_n = scw3_nxt[hb][:, :, 0:O]
                    cv_n = scw3_nxt[hb][:, :, O:2 * O]
                    nc.scalar.activation(sv_n, ph_h, ACT.Sin, bias=b_mpi[:, :], scale=1.0)
                    nc.scalar.activation(tmp3[:, hs, :], ph_h, ACT.Abs,
                                         bias=b_mpi[:, :], scale=1.0)
                    nc.scalar.activation(cv_n, tmp3[:, hs, :], ACT.Sin,
                                         bias=b_hpi[:, :], scale=-1.0)
                else:
                    # final sig = cos(phases), per half (starts AG sooner)
                    nc.scalar.activation(tmp3[:, hs, :], ph_h, ACT.Abs,
                                         bias=b_mpi[:, :], scale=1.0)
                    nc.scalar.activation(cnat3[:, hs, :], tmp3[:, hs, :], ACT.Sin,
                                         bias=b_hpi[:, :], scale=-1.0)

            for step in range(STEPS):
                scw_pair = scws[step % 2]
                scw3 = scw3s[step % 2]
                scw3_nxt = scw3s[(step + 1) % 2]

                def scw_sl(jt):
                    t = scw_pair[jt // HBT]
                    j = jt % HBT
                    return t[:, j * SW:(j + 1) * SW]

                pt = ps1.tile([P, NT * SW], f32, tag="pt", bufs=2)
                pt3 = pt[:, 0:NT * SW].rearrange("p (t w) -> p t w", w=SW)

                # ib0 matmuls
                pc0 = ps2.tile([SW, 512], f32, tag="pc", name="pc0")
                for jt in range(NT):
                    nc.tensor.matmul(
                        pc0[:, :],
                        lhsT=scw_sl(jt),
                        rhs=etb[:, jt * N: jt * N + 512],
                        start=(jt == 0),
                        stop=(jt == NT - 1),
                    )
                ce0 = work.tile([SW, 512], f32, tag="ce0", name="ce0")
                nc.vector.tensor_copy(ce0[:, :], pc0[:, :])
                # ib1 matmuls with ib0's transposes + update woven in: the
                # PE reaches the transposes only after ce0 is long ready, and
                # the DVE/ACT half-0 update overlaps the rest of ib1's MMs
                pc1 = ps2.tile([SW, 512], f32, tag="pc", name="pc1")
                for jt in range(NT):
                    nc.tensor.matmul(
                        pc1[:, :],
                        lhsT=scw_sl(jt),
                        rhs=etb[:, jt * N + 512: jt * N + 1024],
                        start=(jt == 0),
                        stop=(jt == NT - 1),
                    )
                    if jt == 2:
                        for itl in range(HB):
                            nc.tensor.transpose(
                                pt[:, itl * SW:(itl + 1) * SW],
                                ce0[:, itl * P:(itl + 1) * P],
                                ident[0:SW, 0:SW],
                            )
                    if jt == 3:
                        half_update(step, 0, pt3, scw3, scw3_nxt)
                ce1 = work.tile([SW, 512], f32, tag="ce1", name="ce1")
                nc.vector.tensor_copy(ce1[:, :], pc1[:, :])
                for itl in range(HB):
                    it = HB + itl
                    nc.tensor.transpose(
                        pt[:, it * SW:(it + 1) * SW],
                        ce1[:, itl * P:(itl + 1) * P],
                        ident[0:SW, 0:SW],
                    )
                half_update(step, 1, pt3, scw3, scw3_nxt)

            # ---------- sig^T -> AllGather ----------
            identb = const.tile([P, P], bf16)
            nc.vector.tensor_copy(identb[:, :], ident[:, :])
            pst = ps2.tile([O, N], bf16, tag="big")
            for it in range(NT):
                nc.tensor.transpose(
                    pst[:, it * P:(it + 1) * P],
                    cnat[:, it * O:(it + 1) * O],
                    identb[:, :],
                )
            sgt = work.tile([O, N], bf16, tag="sgt2")
            nc.vector.tensor_copy(sgt[:, :], pst[:, :])

            ag_in = dram.tile([O, N], bf16)
            ag_out = dram.tile([H * O, N], bf16)
            nc.sync.dma_start(ag_in[:, :], sgt[:, :])
            nc.gpsimd.collective_compute(
                "AllGather",
                ALU.bypass,
                replica_groups=groups,
                ins=[ag_in[:, :].opt()],
                outs=[ag_out[:, :].opt()],
            )
            sgf = data.tile([H * O + 1, N], bf16)
            nc.sync.dma_start(sgf[0:H * O, :], ag_out[:, :])
            nc.vector.memset(sgf[H * O:H * O + 1, :], 1.0)

            # ---------- output projection ----------
            for it in range(NT):
                po = ps2.tile([P, D], f32, tag="pc")
                nc.tensor.matmul(po[:, :], lhsT=sgf[:, it * P:(it + 1) * P],
                                 rhs=wob_s[:, :], start=True, stop=True)
                ot = work.tile([P, D], f32, tag="ot")
                nc.vector.tensor_copy(ot[:, :], po[:, :])
                nc.sync.dma_start(outp[it * P:(it + 1) * P, :], ot[:, :])

    nc.compile()
    return nc


def get_nc():
    if "nc" not in _CACHE:
        _CACHE["nc"] = _build_nc()
    return _CACHE["nc"]


def make_in_maps(x, Wq, Wk, Wp, bp, Wo, bo, omega, coupling_scale):
    import concourse.mybir as mybir

    bf16 = mybir.dt.np(mybir.dt.bfloat16)
    x = np.asarray(x, np.float32)
    Wq = np.asarray(Wq, np.float32)
    Wk = np.asarray(Wk, np.float32)
    Wp = np.asarray(Wp, np.float32)
    bp = np.asarray(bp, np.float32)
    Wo = np.asarray(Wo, np.float32)
    bo = np.asarray(bo, np.float32)
    omega = np.asarray(omega, np.float32)
    cs = float(np.asarray(coupling_scale, np.float32))

    wob_full = np.ascontiguousarray(
        np.concatenate([Wo.T, bo[None, :]], axis=0)).astype(bf16)
    csdt_full = np.full((P, 1), DT * cs, np.float32)

    in_maps = []
    for c in range(NCORES):
        b, h = c // H, c % H
        in_maps.append({
            "xT": np.ascontiguousarray(x[b].T).astype(bf16),
            "wqT": np.ascontiguousarray(Wq[h * DK:(h + 1) * DK, :].T).astype(bf16),
            "wkT": np.ascontiguousarray(Wk[h * DK:(h + 1) * DK, :].T).astype(bf16),
            "wpT": np.ascontiguousarray(Wp[h * O:(h + 1) * O, :].T).astype(bf16),
            "bpr": np.ascontiguousarray(
                (bp[h * O:(h + 1) * O] + np.pi)[:, None], np.float32),
            "csdt": csdt_full,
            "dtom": np.ascontiguousarray(
                np.tile((DT * omega[h])[None, :], (P, NT)), np.float32),
            "wob": wob_full,
        })
    return in_maps


def run_on_hw(in_maps, trace=False):
    from concourse.bass_utils import run_bass_kernel_spmd

    nc = get_nc()
    return run_bass_kernel_spmd(nc, in_maps, core_ids=list(range(NCORES)), trace=trace)


def kernel(x, Wq, Wk, Wp, bp, Wo, bo, omega, coupling_scale):
    in_maps = make_in_maps(x, Wq, Wk, Wp, bp, Wo, bo, omega, coupling_scale)
    res = run_on_hw(in_maps, trace=False)
    out = np.stack([res.results[0]["out"], res.results[H]["out"]], axis=0)
    return np.ascontiguousarray(out, np.float32)
